# revision 25
# baseline (speedup 1.0000x reference)
"""AffineEdgeAttention Trainium2 kernel (fp16 I/O, DVE-dots).

out[b, i, j] = head[b, i] . w_h + dep[b, j] . w_d + edge_b
with w_h = edge_W[0, :D], w_d = edge_W[0, D:].

Sharding: data-parallel over batch; 16 batches / 8 cores = 2 per core.

All HBM traffic is fp16 (gate is 2e-2; fp16 keeps ~5e-4): per core
3+3 MiB loads + 4 MiB stores ~= 10.25 MiB -> ~29 us floor at the
358 GB/s per-NC HBM limit. Loads stay contiguous: the xbar DMA
transpose runs at ~186 GB/s serialized on its issuing engine and two
concurrent transpose streams corrupt each other, so it is avoided.
GpSimd is left idle: its f16 tensor ops measure ~3.1 us per pair tile
AND slow concurrent DVE ops ~4x (shared SBUF path), so using it is
strictly worse than leaving everything on DVE.

  dots (DVE): one fused scalar_tensor_tensor per 128-row chunk
    (multiply by the w broadcast + free-axis accumulate, 870 ns at 1x)
    -> sd / sh columns (f32). 32 chunks ~= 28 us, streaming right
    behind the loads; this co-paces with the DMA floor.
  s_d broadcast: per column one K=128 matmul with stride-0 stationary
    and IDENTITY rhs transposes+broadcasts into PSUM [128, S]; one ACT
    op folds +edge_b and casts to fp16 SBUF.
  output adds: ACT activation-add for pairs 0-2 (~1.0 us each), DVE
    tensor_scalar_add (4x fp16, ~0.4 us) for the last pair of each
    batch so the tail after the final dot is short. V-pair stores ride
    the sync ring, A-pair stores the scalar ring.
edge_b is baked in at trace time via memset (known host-side).
"""

import sys

import numpy as np

for _p in ("/opt/trn_rl_repo", "/root/.axon_site/_ro/trn_rl_repo"):
    if _p not in sys.path:
        sys.path.insert(0, _p)

import concourse.bacc as bacc
import concourse.bass as bass
import concourse.tile as tile
from concourse import mybir
from concourse.bass_utils import run_bass_kernel_spmd

B, S, D = 16, 1024, 768
N_CORES = 8
BPC = B // N_CORES  # batches per core
P = 128
C = S // P  # 8 row-chunks of 128
NPAIR = C // 2  # 4 chunk-pair tiles per tensor per batch

F16 = mybir.dt.float16
F32 = mybir.dt.float32

# per-pair output engine: V takes the last pair of each batch (short
# tail right after its own dots), ACT the rest.
OUT_PAIR_ENG = ["A", "A", "A", "V"]


def build_program(b_const: float) -> bass.Bass:
    nc = bacc.Bacc("TRN2", target_bir_lowering=False, debug=False)
    head = nc.dram_tensor("head", [BPC, S, D], F16, kind="ExternalInput").ap()
    dep = nc.dram_tensor("dep", [BPC, S, D], F16, kind="ExternalInput").ap()
    w = nc.dram_tensor("edge_W", [1, 2 * D], F16, kind="ExternalInput").ap()
    out = nc.dram_tensor("out", [BPC, S, S], F16, kind="ExternalOutput").ap()

    # [b, t, p, c, d]: chunk-pair t, intra-pair c; rows (2t+c)*128+p
    head_v = head.rearrange("b (t c p) d -> b t p c d", c=2, p=P)
    dep_v = dep.rearrange("b (t c p) d -> b t p c d", c=2, p=P)
    out_v = out.rearrange("b (t c p) j -> b t p c j", c=2, p=P)

    with tile.TileContext(nc) as tc:
        with (
            tc.tile_pool(name="singles", bufs=1) as singles,
            tc.tile_pool(name="loads", bufs=2 * NPAIR) as loads,
            tc.tile_pool(name="svec", bufs=2) as svec,
            tc.tile_pool(name="scratch", bufs=2) as scratch,
            tc.tile_pool(name="bcast", bufs=2) as bcast,
            tc.tile_pool(name="outs", bufs=6) as outs,
            tc.tile_pool(name="psd", bufs=2, space="PSUM") as psd,
            tc.tile_pool(name="psinit", bufs=1, space="PSUM") as psinit,
        ):
            # ---- constants ----
            iota_f = singles.tile([P, P], F32)
            nc.gpsimd.iota(
                iota_f, [[1, P]], channel_multiplier=0,
                allow_small_or_imprecise_dtypes=True,
            )
            iota_p = singles.tile([P, 1], F32)
            nc.gpsimd.iota(
                iota_p, [[0, 1]], channel_multiplier=1,
                allow_small_or_imprecise_dtypes=True,
            )
            ident = singles.tile([P, P], F32)
            nc.vector.tensor_scalar(
                out=ident, in0=iota_f, scalar1=iota_p, scalar2=None,
                op0=mybir.AluOpType.is_equal,
            )
            # w rides FIRST on the sync ring (3 KB, delays the dep loads
            # by ~0.1 us but starts ~6 us earlier than the scalar ring,
            # whose first dispatch sits behind the ACT preamble)
            w_row = singles.tile([1, 2 * D], F16)
            nc.sync.dma_start(out=w_row, in_=w)
            ones = singles.tile([1, P], F16)
            nc.vector.memset(ones, 1.0)
            bt = singles.tile([P, 1], F32)
            nc.vector.memset(bt, b_const)

            # w_h / w_d broadcast to all 128 partitions, fp16 SBUF
            psw_d = psinit.tile([P, D], F32)
            psw_h = psinit.tile([P, D], F32)
            for dst, lo in ((psw_d, D), (psw_h, 0)):
                for k0, k1 in ((0, 512), (512, D)):  # psum bank boundary
                    nc.tensor.matmul(
                        dst[:, k0:k1],
                        lhsT=ones,
                        rhs=w_row[:, lo + k0 : lo + k1],
                        start=True,
                        stop=True,
                    )
            # evacuate on V (no ACT table-load gate): V's stream opens
            # with these two copies, then streams straight into the dots
            wtd = singles.tile([P, D], F16)
            nc.vector.tensor_copy(wtd, psw_d)
            wth = singles.tile([P, D], F16)
            nc.vector.tensor_copy(wth, psw_h)

            # ---- all loads up front on the sync ring; dep first per
            # batch (its chain to sdb is longer) ----
            dep_tiles = []
            head_tiles = []
            for bi in range(BPC):
                dep_t = []
                for t in range(NPAIR):
                    dt_ = loads.tile([P, 2, D], F16, tag="dep")
                    nc.sync.dma_start(out=dt_, in_=dep_v[bi, t])
                    dep_t.append(dt_)
                head_t = []
                for t in range(NPAIR):
                    ht = loads.tile([P, 2, D], F16, tag="head")
                    nc.sync.dma_start(out=ht, in_=head_v[bi, t])
                    head_t.append(ht)
                dep_tiles.append(dep_t)
                head_tiles.append(head_t)

            def fused_dot(src_chunk, w_tile, acc_col):
                """acc_col[p] = sum_d src_chunk[p, d] * w_tile[p, d] (DVE)."""
                prod = scratch.tile([P, D], F16, tag="prodV", name="prod")
                nc.vector.scalar_tensor_tensor(
                    out=prod,
                    in0=src_chunk,
                    scalar=0.0,
                    in1=w_tile,
                    op0=mybir.AluOpType.bypass,
                    op1=mybir.AluOpType.mult,
                    accum_out=acc_col,
                )

            for bi in range(BPC):
                # ---- s_d columns (DVE) -> PE transpose+broadcast ----
                sd = svec.tile([P, C], F32, tag="sd")
                ps = psd.tile([P, S], F32, tag="ps")
                for t in range(NPAIR):
                    for i in range(2):
                        c = 2 * t + i
                        fused_dot(
                            dep_tiles[bi][t][:, i, :], wtd, sd[:, c : c + 1]
                        )
                    for k in (2 * t, 2 * t + 1):
                        nc.tensor.matmul(
                            ps[:, k * P : (k + 1) * P],
                            lhsT=sd[:, k : k + 1].broadcast_to((P, P)),
                            rhs=ident,
                            start=True,
                            stop=True,
                        )
                sdb = bcast.tile([P, S], F16, tag="sdb")
                nc.scalar.add(out=sdb, in_=ps, add=bt)

                # ---- s_h chunks + output chunks ----
                sh = svec.tile([P, C], F32, tag="sh")
                for t in range(NPAIR):
                    for i in range(2):
                        c = 2 * t + i
                        fused_dot(
                            head_tiles[bi][t][:, i, :], wth, sh[:, c : c + 1]
                        )
                    ot = outs.tile([P, 2, S], F16, tag="ot")
                    for i in range(2):
                        c = 2 * t + i
                        if OUT_PAIR_ENG[t] == "A":
                            nc.scalar.add(
                                out=ot[:, i, :], in_=sdb, add=sh[:, c : c + 1]
                            )
                        else:
                            nc.vector.tensor_scalar_add(
                                ot[:, i, :], sdb, sh[:, c : c + 1]
                            )
                    if OUT_PAIR_ENG[t] == "A":
                        nc.scalar.dma_start(out=out_v[bi, t], in_=ot)
                    else:
                        nc.sync.dma_start(out=out_v[bi, t], in_=ot)
    nc.compile()
    return nc


def kernel(head, dep, edge_W, edge_b, _trace=False):
    nc = build_program(float(edge_b[0]))
    head16 = head.astype(np.float16)
    dep16 = dep.astype(np.float16)
    w16 = edge_W.astype(np.float16)
    in_maps = []
    for k in range(N_CORES):
        in_maps.append(
            {
                "head": np.ascontiguousarray(head16[k * BPC : (k + 1) * BPC]),
                "dep": np.ascontiguousarray(dep16[k * BPC : (k + 1) * BPC]),
                "edge_W": w16,
            }
        )
    res = run_bass_kernel_spmd(nc, in_maps, core_ids=list(range(N_CORES)), trace=_trace)
    out = np.concatenate([r["out"] for r in res.results], axis=0).astype(np.float32)
    if _trace:
        return out, res
    return out


if __name__ == "__main__":
    rng = np.random.default_rng(0)
    head = rng.standard_normal((B, S, D), dtype=np.float32)
    dep = rng.standard_normal((B, S, D), dtype=np.float32)
    edge_W = rng.standard_normal((1, 2 * D), dtype=np.float32)
    edge_b = rng.standard_normal((1,), dtype=np.float32)
    out = kernel(head, dep, edge_W, edge_b)
    ref = (
        head @ edge_W[0, :D]
    )[:, :, None] + (dep @ edge_W[0, D:])[:, None, :] + edge_b[0]
    err = np.abs(out - ref).max() / np.abs(ref).max()
    print("max rel err:", err)
